# revision 18
# baseline (speedup 1.0000x reference)
"""Trainium2 Bass kernel for nn_DelayedHORN.

Reference semantics (per timestep t, state x,y in R^{B x N}, N=128):
    I   = s[:,t] * w_ih + b_ih + y_{t-4} @ w_hh.T + b_hh
(The reference's 5-slot rolling buffer receives y_{t+1} at the END of step
t, so the value read at step t is y_{t-4}: the effective delay is 4.)
    th  = tanh(I / sqrt(N))
    x'  = x + y
    y'  = y + th - 0.2*y - x
Output: x_T @ w_ro.T + b_ro.

Reformulation used on device (exact in real arithmetic):
    e_t = th_t - th_{t-1}            (th_{-1} := 0)
    y_{t+1} = 1.8*y_t - 1.8*y_{t-1} + e_t   (y_0 = y_{-1} = 0)
    x_T = sum_{t<T} y_t
The delay lets us compute 5 timesteps of tanh with a single pair of
matmuls: steps 5g..5g+4 need y_{5g-4}..y_{5g}, which is exactly the block
of chain outputs produced by group g-1 (stored contiguously), plus the
rank-1 w_ih (x) s outer product accumulated in PSUM.  The serial per-step
work is one scalar_tensor_tensor on the vector engine.

Sharding: pure data parallel, batch 512 -> 64 per core across 8 cores.
Layout on device: [n=128 partitions, b=64 free].
"""

import numpy as np
from contextlib import ExitStack

N = 128
BL = 64          # batch per core
NCORES = 8
T_FULL = 784
CHUNK_G = 32     # groups of 5 timesteps per DMA chunk of the input sequence


def _build(T, use_f32r=True):
    import concourse.bass as bass
    import concourse.mybir as mybir
    import concourse.tile as tile

    f32 = mybir.dt.float32
    f32r = mybir.dt.float32r if use_f32r else mybir.dt.float32
    Alu = mybir.AluOpType
    Act = mybir.ActivationFunctionType

    n_sub = T - 1                      # y_1 .. y_{T-1}
    NG = (n_sub + 4) // 5              # groups of 5 sub-steps
    LAST_SUBS = n_sub - 5 * (NG - 1)   # 1..5 sub-steps in the last group
    W = 5 * BL                         # 320: free width of one group block

    nc = bass.Bass("TRN2", debug=False, enable_asserts=False, num_devices=NCORES)

    # ---- DRAM I/O ----
    # pack1: [w_hhT | ident | w_roT | biasc] along free dim; pack2: w_ih row + b_ro col
    s_dram = nc.dram_tensor("s_seq", [1, T * BL], f32r, kind="ExternalInput").ap()
    pk1_dram = nc.dram_tensor("pack1", [N, 651], f32r, kind="ExternalInput").ap()
    pk2_dram = nc.dram_tensor("pack2", [N, 129], f32r, kind="ExternalInput").ap()
    out_dram = nc.dram_tensor("out", [10, BL], f32, kind="ExternalOutput").ap()

    with tile.TileContext(nc) as tc, ExitStack() as ctx:
        const_pool = ctx.enter_context(tc.tile_pool(name="const", bufs=1))
        state_pool = ctx.enter_context(tc.tile_pool(name="state", bufs=1))
        s_pool = ctx.enter_context(tc.tile_pool(name="schunk", bufs=2))
        psum_pool = ctx.enter_context(
            tc.tile_pool(name="psum", bufs=2, space="PSUM")
        )
        xacc_pool = ctx.enter_context(
            tc.tile_pool(name="xacc", bufs=1, space="PSUM")
        )

        # ---- constants (2 packed DMAs to stay under PE wait-slot limits) ----
        pk1 = const_pool.tile([N, 651], f32r)
        pk2 = const_pool.tile([N, 129], f32r)
        nc.sync.dma_start(pk1[:], pk1_dram)
        nc.sync.dma_start(pk2[:], pk2_dram)
        whh = pk1[:, 0:128]
        ident = pk1[:, 128:256]
        wro = pk1[:, 256:266].bitcast(f32)
        biasc = pk1[:, 266:267].bitcast(f32)
        wih = pk2[0:1, 0:128]
        bro = pk2[0:10, 128:129].bitcast(f32)

        # ---- per-group pool tiles ----
        # R_g holds group g's chain outputs [y_{5g+1} .. y_{5g+5}]; group g+1's
        # recurrent matmul consumes R_g wholesale (delay-4 window is exactly
        # the previous group's outputs).
        r_pool = ctx.enter_context(tc.tile_pool(name="r", bufs=3))
        th_pool = ctx.enter_context(tc.tile_pool(name="th", bufs=2))
        e_pool = ctx.enter_context(tc.tile_pool(name="e", bufs=2))
        f_pool = ctx.enter_context(tc.tile_pool(name="f", bufs=2))
        th0 = state_pool.tile([N, W], f32, name="th_init")
        rinit = state_pool.tile([N, W], f32, name="rinit")
        xacc_ps = xacc_pool.tile([N, W], f32)
        xacc_sb = state_pool.tile([N, BL], f32)
        out_ps = xacc_pool.tile([10, BL], f32)
        out_sb = state_pool.tile([10, BL], f32)
        # scratch PSUM target for 1x1 "touch" matmuls: PE matmuls only have a
        # single HW sync-wait slot, so a touch matmul absorbs each DMA-queue
        # semaphore before the real matmuls need the data
        touch_ps = xacc_pool.tile([1, 1], f32)

        def pe_touch(ap_1x1):
            a = ap_1x1.bitcast(f32)
            nc.tensor.matmul(
                touch_ps[:], a, a, start=True, stop=True,
                skip_group_check=True,
            )

        # zero-init: R_{-1} = [y_{-4}..y_0] = 0 and th_{-1} = 0.
        # rinit zeros come via DMA from pack1's zero region: walrus rejects
        # float32r Memset (ISA check), and the fp32r-matmul verifier rejects
        # f32-Memset producers; DMACopy with f32r output passes both.
        r32 = (lambda ap: ap.bitcast(f32r)) if use_f32r else (lambda ap: ap)
        nc.sync.dma_start(r32(rinit[:]), pk1_dram[:, 267:587])
        nc.vector.memset(th0[:, 4 * BL : W], 0.0)

        inv = float(1.0 / np.sqrt(N))

        schunk = None
        thp = th0
        rprev = rinit
        for g in range(NG):
            subs = 5 if g < NG - 1 else LAST_SUBS
            w = subs * BL  # th/e/f width needed this group
            thc = th_pool.tile([N, W], f32, name="th")
            E = e_pool.tile([N, W], f32, name="E")
            F = f_pool.tile([N, W], f32, name="F")
            R = r_pool.tile([N, W], f32, name="R")

            # stream the input sequence in chunks (rows 5g..: CHUNK_G groups)
            if g % CHUNK_G == 0:
                r0 = 5 * g * BL
                r1 = min(T * BL, (5 * g + 5 * CHUNK_G) * BL)
                schunk = s_pool.tile([1, 5 * CHUNK_G * BL], f32r)
                nc.sync.dma_start(schunk[:1, 0 : r1 - r0], s_dram[0:1, r0:r1])
                if g == 0:
                    pe_touch(pk1[0:1, 0:1])
                    pe_touch(pk2[0:1, 0:1])
                pe_touch(schunk[0:1, 0:1])
            soff = (g % CHUNK_G) * W

            # ---- PE: I = w_ih (x) s + w_hh @ y_delayed  (PSUM accumulate) ----
            # rank-1 matmul first: it carries the DMA-chunk + psum-WAR waits,
            # the w_hh matmul then only waits on the DVE chain.
            mm = psum_pool.tile([N, W], f32)
            nc.tensor.matmul(
                mm[:, 0:w],
                wih,
                schunk[0:1, soff : soff + w],
                start=True,
                stop=False,
            )
            nc.tensor.matmul(
                mm[:, 0:w], whh, r32(rprev[:, 0:w]), start=False, stop=True
            )

            # ---- ACT: th = tanh(inv * I + biasc) ----
            nc.scalar.activation(
                thc[:, 0:w], mm[:, 0:w], Act.Tanh, bias=biasc, scale=inv
            )

            # ---- Pool: e_t = th_t - th_{t-1} ----
            nc.gpsimd.tensor_sub(E[:, 0:BL], thc[:, 0:BL], thp[:, 4 * BL : W])
            if w > BL:
                nc.gpsimd.tensor_sub(E[:, BL:w], thc[:, BL:w], thc[:, 0 : w - BL])

            # ---- sub-steps: J produces y_{5g+J+1} -> R block J ----
            for J in range(subs):
                lo = J * BL
                # y_{t-1} = y_{5g+J-1}
                if J == 0:
                    ym1 = rprev[:, 3 * BL : 4 * BL]
                elif J == 1:
                    ym1 = rprev[:, 4 * BL : W]
                else:
                    ym1 = R[:, (J - 2) * BL : (J - 1) * BL]
                # y_t = y_{5g+J}
                yt = rprev[:, 4 * BL : W] if J == 0 else R[:, (J - 1) * BL : J * BL]
                ydst = R[:, lo : lo + BL]

                # DVE: f_t = -1.8*y_{t-1} + e_t  (scalar_tensor_tensor is
                # DVE-only in this walrus build)
                nc.vector.scalar_tensor_tensor(
                    F[:, lo : lo + BL], ym1, -1.8, E[:, lo : lo + BL],
                    op0=Alu.mult, op1=Alu.add,
                )
                # DVE (serial chain): y_{t+1} = 1.8*y_t + f_t
                # (write as f32r: PE consumes R in single-pass fp32r mode)
                nc.vector.scalar_tensor_tensor(
                    r32(ydst), yt, 1.8, F[:, lo : lo + BL],
                    op0=Alu.mult, op1=Alu.add,
                )

            thp = thc

            # ---- PE: x accumulation, xacc_ps[:, J*BL+b] += y_{5g+J+1}[b] ----
            nc.tensor.matmul(
                xacc_ps[:, 0:w],
                ident,
                r32(R[:, 0:w]),
                start=(g == 0),
                stop=(g == NG - 1),
                skip_group_check=True,
            )
            rprev = R

        # ---- epilogue: x_T = fold(xacc), out = w_ro @ x_T + b_ro ----
        nc.vector.tensor_reduce(
            xacc_sb[:],
            xacc_ps[:].rearrange("p (j b) -> p b j", j=5),
            axis=mybir.AxisListType.X,
            op=Alu.add,
        )
        nc.tensor.matmul(out_ps[:], wro, xacc_sb[:], start=True, stop=True)
        nc.vector.tensor_scalar(
            out_sb[:], out_ps[:], bro, None, op0=Alu.add
        )
        nc.sync.dma_start(out_dram, out_sb[:])

    _split_excess_waits(nc)
    return nc


def _split_excess_waits(nc):
    """This toolchain's walrus codegen accepts at most ONE semaphore wait per
    engine instruction ("Too many sync wait commands").  Tile emits up to ~3.
    Hoist all but the last wait of each instruction onto same-engine NoOps
    placed immediately before it -- the engine executes them in order, so the
    happens-before relation is preserved."""
    import concourse.mybir as mybir

    k = 0
    for fn in nc.m.functions:
        for bb in fn.blocks:
            insts = bb.instructions
            out = []
            changed = False
            for inst in insts:
                si = inst.sync_info
                waits = list(si.on_wait) if (si is not None and si.on_wait) else []
                if len(waits) > 1:
                    for wsub in waits[:-1]:
                        k += 1
                        nop = mybir.InstNoOp(name=f"WSPLIT-{k}", ins=[], outs=[])
                        nop.engine = inst.engine
                        nop.sync_info = mybir.SyncInfo(on_wait=[wsub], on_update=[])
                        out.append(nop)
                    inst.sync_info = mybir.SyncInfo(
                        on_wait=[waits[-1]], on_update=list(si.on_update or [])
                    )
                    changed = True
                out.append(inst)
            if changed:
                bb.instructions[:] = out


_BUILD_CACHE = {}


def _get_nc(T, use_f32r=True):
    key = (T, use_f32r)
    if key not in _BUILD_CACHE:
        _BUILD_CACHE[key] = _build(T, use_f32r)
    return _BUILD_CACHE[key]


def _run(input_sequence, w_ih, b_ih, w_hh, b_hh, w_ro, b_ro, T=None, trace=False, use_f32r=True):
    from concourse.bass_utils import run_bass_kernel_spmd

    input_sequence = np.asarray(input_sequence, np.float32)
    B, T_in = input_sequence.shape
    if T is None:
        T = T_in
    assert B == BL * NCORES
    nc = _get_nc(T, use_f32r)

    inv = np.float32(1.0 / np.sqrt(N))
    biasc_col = (
        (np.asarray(b_ih, np.float32) + np.asarray(b_hh, np.float32)) * inv
    )[:, None]
    pack1 = np.concatenate(
        [
            np.asarray(w_hh, np.float32).T,
            np.eye(N, dtype=np.float32),
            np.asarray(w_ro, np.float32).T,
            biasc_col,
            np.zeros((N, 384), np.float32),
        ],
        axis=1,
    ).astype(np.float32)
    pack2 = np.zeros((N, 129), np.float32)
    pack2[0, 0:128] = np.asarray(w_ih, np.float32)
    pack2[0:10, 128] = np.asarray(b_ro, np.float32)
    common = {"pack1": np.ascontiguousarray(pack1), "pack2": np.ascontiguousarray(pack2)}
    in_maps = []
    for i in range(NCORES):
        s_i = np.ascontiguousarray(
            input_sequence[i * BL : (i + 1) * BL, :T].T
        )  # [T, BL]
        m = dict(common)
        m["s_seq"] = s_i.reshape(1, T * BL)
        in_maps.append(m)

    res = run_bass_kernel_spmd(
        nc, in_maps, core_ids=list(range(NCORES)), trace=trace
    )
    outs = [r["out"] for r in res.results]  # each [10, 64]
    full = np.concatenate(outs, axis=1).T.astype(np.float32)  # [512, 10]
    return full, res


def kernel(input_sequence, w_ih, b_ih, w_hh, b_hh, w_ro, b_ro):
    out, _ = _run(input_sequence, w_ih, b_ih, w_hh, b_hh, w_ro, b_ro)
    return out


# revision 20
# speedup vs baseline: 1.4302x; 1.4302x over previous
"""Trainium2 Bass kernel for nn_DelayedHORN.

Reference semantics (per timestep t, state x,y in R^{B x N}, N=128):
    I   = s[:,t] * w_ih + b_ih + y_{t-4} @ w_hh.T + b_hh
(The reference's 5-slot rolling buffer receives y_{t+1} at the END of step
t, so the value read at step t is y_{t-4}: the effective delay is 4.)
    th  = tanh(I / sqrt(N))
    x'  = x + y
    y'  = y + th - 0.2*y - x
Output: x_T @ w_ro.T + b_ro.

Reformulation used on device (exact in real arithmetic):
    e_t = th_t - th_{t-1}            (th_{-1} := 0)
    y_{t+1} = 1.8*y_t - 1.8*y_{t-1} + e_t   (y_0 = y_{-1} = 0)
    x_T = sum_{t<T} y_t
The delay lets us compute 5 timesteps of tanh with a single pair of
matmuls: steps 5g..5g+4 need y_{5g-4}..y_{5g}, which is exactly the block
of chain outputs produced by group g-1 (stored contiguously), plus the
rank-1 w_ih (x) s outer product accumulated in PSUM.  The serial per-step
work is one scalar_tensor_tensor on the vector engine.

Sharding: pure data parallel, batch 512 -> 64 per core across 8 cores.
Layout on device: [n=128 partitions, b=64 free].
"""

import numpy as np
from contextlib import ExitStack

N = 128
BL = 64          # batch per core
NCORES = 8
T_FULL = 784
CHUNK_G = 32     # groups of 5 timesteps per DMA chunk of the input sequence


def _build(T, use_f32r=True):
    import concourse.bass as bass
    import concourse.mybir as mybir
    import concourse.tile as tile

    f32 = mybir.dt.float32
    f32r = mybir.dt.float32r if use_f32r else mybir.dt.float32
    Alu = mybir.AluOpType
    Act = mybir.ActivationFunctionType

    n_sub = T - 1                      # y_1 .. y_{T-1}
    NG = (n_sub + 4) // 5              # groups of 5 sub-steps
    LAST_SUBS = n_sub - 5 * (NG - 1)   # 1..5 sub-steps in the last group
    W = 5 * BL                         # 320: free width of one group block

    nc = bass.Bass("TRN2", debug=False, enable_asserts=False, num_devices=NCORES)

    # ---- DRAM I/O ----
    # pack1: [w_hhT | ident | w_roT | biasc] along free dim; pack2: w_ih row + b_ro col
    s_dram = nc.dram_tensor("s_seq", [1, T * BL], f32r, kind="ExternalInput").ap()
    pk1_dram = nc.dram_tensor("pack1", [N, 651], f32r, kind="ExternalInput").ap()
    pk2_dram = nc.dram_tensor("pack2", [N, 129], f32r, kind="ExternalInput").ap()
    out_dram = nc.dram_tensor("out", [10, BL], f32, kind="ExternalOutput").ap()

    with tile.TileContext(nc) as tc, ExitStack() as ctx:
        const_pool = ctx.enter_context(tc.tile_pool(name="const", bufs=1))
        state_pool = ctx.enter_context(tc.tile_pool(name="state", bufs=1))
        s_pool = ctx.enter_context(tc.tile_pool(name="schunk", bufs=2))
        psum_pool = ctx.enter_context(
            tc.tile_pool(name="psum", bufs=3, space="PSUM")
        )
        xacc_pool = ctx.enter_context(
            tc.tile_pool(name="xacc", bufs=1, space="PSUM")
        )

        # ---- constants (2 packed DMAs to stay under PE wait-slot limits) ----
        pk1 = const_pool.tile([N, 651], f32r)
        pk2 = const_pool.tile([N, 129], f32r)
        nc.sync.dma_start(pk1[:], pk1_dram)
        nc.sync.dma_start(pk2[:], pk2_dram)
        whh = pk1[:, 0:128]
        ident = pk1[:, 128:256]
        wro = pk1[:, 256:266].bitcast(f32)
        biasc = pk1[:, 266:267].bitcast(f32)
        wih = pk2[0:1, 0:128]
        bro = pk2[0:10, 128:129].bitcast(f32)

        # ---- per-group pool tiles ----
        # R_g holds group g's chain outputs [y_{5g+1} .. y_{5g+5}]; group g+1's
        # recurrent matmul consumes R_g wholesale (delay-4 window is exactly
        # the previous group's outputs).
        r_pool = ctx.enter_context(tc.tile_pool(name="r", bufs=3))
        th_pool = ctx.enter_context(tc.tile_pool(name="th", bufs=3))
        e_pool = ctx.enter_context(tc.tile_pool(name="e", bufs=3))
        f_pool = ctx.enter_context(tc.tile_pool(name="f", bufs=3))
        th0 = state_pool.tile([N, W], f32, name="th_init")
        rinit = state_pool.tile([N, W], f32, name="rinit")
        xacc_ps = xacc_pool.tile([N, W], f32)
        xacc_sb = state_pool.tile([N, BL], f32)
        out_ps = xacc_pool.tile([10, BL], f32)
        out_sb = state_pool.tile([10, BL], f32)
        # scratch PSUM target for 1x1 "touch" matmuls: PE matmuls only have a
        # single HW sync-wait slot, so a touch matmul absorbs each DMA-queue
        # semaphore before the real matmuls need the data
        touch_ps = xacc_pool.tile([1, 1], f32)

        def pe_touch(ap_1x1):
            a = ap_1x1.bitcast(f32)
            nc.tensor.matmul(
                touch_ps[:], a, a, start=True, stop=True,
                skip_group_check=True,
            )

        # zero-init: R_{-1} = [y_{-4}..y_0] = 0 and th_{-1} = 0.
        # rinit zeros come via DMA from pack1's zero region: walrus rejects
        # float32r Memset (ISA check), and the fp32r-matmul verifier rejects
        # f32-Memset producers; DMACopy with f32r output passes both.
        r32 = (lambda ap: ap.bitcast(f32r)) if use_f32r else (lambda ap: ap)
        nc.sync.dma_start(r32(rinit[:]), pk1_dram[:, 267:587])
        nc.vector.memset(th0[:, 4 * BL : W], 0.0)

        inv = float(1.0 / np.sqrt(N))

        schunk = None
        thp = th0
        rprev = rinit
        for g in range(NG):
            subs = 5 if g < NG - 1 else LAST_SUBS
            w = subs * BL  # th/e/f width needed this group
            thc = th_pool.tile([N, W], f32, name="th")
            E = e_pool.tile([N, W], f32, name="E")
            F = f_pool.tile([N, W], f32, name="F")
            R = r_pool.tile([N, W], f32, name="R")

            # stream the input sequence in chunks (rows 5g..: CHUNK_G groups)
            if g % CHUNK_G == 0:
                r0 = 5 * g * BL
                r1 = min(T * BL, (5 * g + 5 * CHUNK_G) * BL)
                schunk = s_pool.tile([1, 5 * CHUNK_G * BL], f32r)
                nc.sync.dma_start(schunk[:1, 0 : r1 - r0], s_dram[0:1, r0:r1])
                if g == 0:
                    pe_touch(pk1[0:1, 0:1])
                    pe_touch(pk2[0:1, 0:1])
                pe_touch(schunk[0:1, 0:1])
            soff = (g % CHUNK_G) * W

            # ---- PE: I = w_ih (x) s + w_hh @ y_delayed  (PSUM accumulate) ----
            # Rank-1 matmul covers the whole group at once (depends only on
            # the DMA'd input chunk).  The w_hh matmul is split per 64-column
            # block: block J only needs R_{g-1}[J] (chain output J of the
            # PREVIOUS group), so the tanh/e/f pipeline for group g streams
            # behind group g-1's chain instead of serializing after it.
            mm = psum_pool.tile([N, W], f32)
            nc.tensor.matmul(
                mm[:, 0:w],
                wih,
                schunk[0:1, soff : soff + w],
                start=True,
                stop=False,
                skip_group_check=True,
            )
            for J in range(subs):
                lo = J * BL
                nc.tensor.matmul(
                    mm[:, lo : lo + BL],
                    whh,
                    r32(rprev[:, lo : lo + BL]),
                    start=False,
                    stop=True,
                    skip_group_check=True,
                )
                # ---- ACT: th_J = tanh(inv * I_J + biasc) ----
                nc.scalar.activation(
                    thc[:, lo : lo + BL], mm[:, lo : lo + BL], Act.Tanh,
                    bias=biasc, scale=inv,
                )
                # ---- Pool: e_J = th_J - th_{J-1} ----
                if J == 0:
                    nc.gpsimd.tensor_sub(
                        E[:, 0:BL], thc[:, 0:BL], thp[:, 4 * BL : W]
                    )
                else:
                    nc.gpsimd.tensor_sub(
                        E[:, lo : lo + BL], thc[:, lo : lo + BL],
                        thc[:, lo - BL : lo],
                    )

            # ---- sub-steps: J produces y_{5g+J+1} -> R block J ----
            for J in range(subs):
                lo = J * BL
                # y_{t-1} = y_{5g+J-1}
                if J == 0:
                    ym1 = rprev[:, 3 * BL : 4 * BL]
                elif J == 1:
                    ym1 = rprev[:, 4 * BL : W]
                else:
                    ym1 = R[:, (J - 2) * BL : (J - 1) * BL]
                # y_t = y_{5g+J}
                yt = rprev[:, 4 * BL : W] if J == 0 else R[:, (J - 1) * BL : J * BL]
                ydst = R[:, lo : lo + BL]

                # DVE: f_t = -1.8*y_{t-1} + e_t  (scalar_tensor_tensor is
                # DVE-only in this walrus build)
                nc.vector.scalar_tensor_tensor(
                    F[:, lo : lo + BL], ym1, -1.8, E[:, lo : lo + BL],
                    op0=Alu.mult, op1=Alu.add,
                )
                # DVE (serial chain): y_{t+1} = 1.8*y_t + f_t
                # (write as f32r: PE consumes R in single-pass fp32r mode)
                nc.vector.scalar_tensor_tensor(
                    r32(ydst), yt, 1.8, F[:, lo : lo + BL],
                    op0=Alu.mult, op1=Alu.add,
                )

            thp = thc

            # ---- PE: x accumulation, xacc_ps[:, J*BL+b] += y_{5g+J+1}[b] ----
            nc.tensor.matmul(
                xacc_ps[:, 0:w],
                ident,
                r32(R[:, 0:w]),
                start=(g == 0),
                stop=(g == NG - 1),
                skip_group_check=True,
            )
            rprev = R

        # ---- epilogue: x_T = fold(xacc), out = w_ro @ x_T + b_ro ----
        nc.vector.tensor_reduce(
            xacc_sb[:],
            xacc_ps[:].rearrange("p (j b) -> p b j", j=5),
            axis=mybir.AxisListType.X,
            op=Alu.add,
        )
        nc.tensor.matmul(out_ps[:], wro, xacc_sb[:], start=True, stop=True)
        nc.vector.tensor_scalar(
            out_sb[:], out_ps[:], bro, None, op0=Alu.add
        )
        nc.sync.dma_start(out_dram, out_sb[:])

    _split_excess_waits(nc)
    return nc


def _split_excess_waits(nc):
    """This toolchain's walrus codegen accepts at most ONE semaphore wait per
    engine instruction ("Too many sync wait commands").  Tile emits up to ~3.
    Hoist all but the last wait of each instruction onto same-engine NoOps
    placed immediately before it -- the engine executes them in order, so the
    happens-before relation is preserved."""
    import concourse.mybir as mybir

    k = 0
    for fn in nc.m.functions:
        for bb in fn.blocks:
            insts = bb.instructions
            out = []
            changed = False
            for inst in insts:
                si = inst.sync_info
                waits = list(si.on_wait) if (si is not None and si.on_wait) else []
                if len(waits) > 1:
                    for wsub in waits[:-1]:
                        k += 1
                        nop = mybir.InstNoOp(name=f"WSPLIT-{k}", ins=[], outs=[])
                        nop.engine = inst.engine
                        nop.sync_info = mybir.SyncInfo(on_wait=[wsub], on_update=[])
                        out.append(nop)
                    inst.sync_info = mybir.SyncInfo(
                        on_wait=[waits[-1]], on_update=list(si.on_update or [])
                    )
                    changed = True
                out.append(inst)
            if changed:
                bb.instructions[:] = out


_BUILD_CACHE = {}


def _get_nc(T, use_f32r=True):
    key = (T, use_f32r)
    if key not in _BUILD_CACHE:
        _BUILD_CACHE[key] = _build(T, use_f32r)
    return _BUILD_CACHE[key]


def _run(input_sequence, w_ih, b_ih, w_hh, b_hh, w_ro, b_ro, T=None, trace=False, use_f32r=True):
    from concourse.bass_utils import run_bass_kernel_spmd

    input_sequence = np.asarray(input_sequence, np.float32)
    B, T_in = input_sequence.shape
    if T is None:
        T = T_in
    assert B == BL * NCORES
    nc = _get_nc(T, use_f32r)

    inv = np.float32(1.0 / np.sqrt(N))
    biasc_col = (
        (np.asarray(b_ih, np.float32) + np.asarray(b_hh, np.float32)) * inv
    )[:, None]
    pack1 = np.concatenate(
        [
            np.asarray(w_hh, np.float32).T,
            np.eye(N, dtype=np.float32),
            np.asarray(w_ro, np.float32).T,
            biasc_col,
            np.zeros((N, 384), np.float32),
        ],
        axis=1,
    ).astype(np.float32)
    pack2 = np.zeros((N, 129), np.float32)
    pack2[0, 0:128] = np.asarray(w_ih, np.float32)
    pack2[0:10, 128] = np.asarray(b_ro, np.float32)
    common = {"pack1": np.ascontiguousarray(pack1), "pack2": np.ascontiguousarray(pack2)}
    in_maps = []
    for i in range(NCORES):
        s_i = np.ascontiguousarray(
            input_sequence[i * BL : (i + 1) * BL, :T].T
        )  # [T, BL]
        m = dict(common)
        m["s_seq"] = s_i.reshape(1, T * BL)
        in_maps.append(m)

    res = run_bass_kernel_spmd(
        nc, in_maps, core_ids=list(range(NCORES)), trace=trace
    )
    outs = [r["out"] for r in res.results]  # each [10, 64]
    full = np.concatenate(outs, axis=1).T.astype(np.float32)  # [512, 10]
    return full, res


def kernel(input_sequence, w_ih, b_ih, w_hh, b_hh, w_ro, b_ro):
    out, _ = _run(input_sequence, w_ih, b_ih, w_hh, b_hh, w_ro, b_ro)
    return out


# revision 21
# speedup vs baseline: 1.5219x; 1.0641x over previous
"""Trainium2 Bass kernel for nn_DelayedHORN.

Reference semantics (per timestep t, state x,y in R^{B x N}, N=128):
    I   = s[:,t] * w_ih + b_ih + y_{t-4} @ w_hh.T + b_hh
(The reference's 5-slot rolling buffer receives y_{t+1} at the END of step
t, so the value read at step t is y_{t-4}: the effective delay is 4.)
    th  = tanh(I / sqrt(N))
    x'  = x + y
    y'  = y + th - 0.2*y - x
Output: x_T @ w_ro.T + b_ro.

Reformulation used on device (exact in real arithmetic):
    e_t = th_t - th_{t-1}            (th_{-1} := 0)
    y_{t+1} = 1.8*y_t - 1.8*y_{t-1} + e_t   (y_0 = y_{-1} = 0)
    x_T = sum_{t<T} y_t
The delay lets us compute 5 timesteps of tanh with a single pair of
matmuls: steps 5g..5g+4 need y_{5g-4}..y_{5g}, which is exactly the block
of chain outputs produced by group g-1 (stored contiguously), plus the
rank-1 w_ih (x) s outer product accumulated in PSUM.  The serial per-step
work is one scalar_tensor_tensor on the vector engine.

Sharding: pure data parallel, batch 512 -> 64 per core across 8 cores.
Layout on device: [n=128 partitions, b=64 free].
"""

import numpy as np
from contextlib import ExitStack

N = 128
BL = 64          # batch per core
NCORES = 8
T_FULL = 784
CHUNK_G = 32     # groups of 5 timesteps per DMA chunk of the input sequence


def _patch_ldw_opt():
    """Enable walrus's LDWEIGHTS dedupe: the per-block w_hh matmul split
    reloads identical stationary weights 5x per group (~290 ns each)
    without it."""
    from concourse import bass_utils as BU

    if getattr(BU, "_ldw_opt_patched", False):
        return
    orig = BU.run_command

    def run2(argv, **kw):
        argv = [
            "--enable-ldw-opt=true" if a == "--enable-ldw-opt=false" else a
            for a in argv
        ]
        return orig(argv, **kw)

    BU.run_command = run2
    BU._ldw_opt_patched = True


def _build(T, use_f32r=True):
    import concourse.bass as bass
    import concourse.mybir as mybir
    import concourse.tile as tile

    _patch_ldw_opt()

    f32 = mybir.dt.float32
    f32r = mybir.dt.float32r if use_f32r else mybir.dt.float32
    Alu = mybir.AluOpType
    Act = mybir.ActivationFunctionType

    n_sub = T - 1                      # y_1 .. y_{T-1}
    NG = (n_sub + 4) // 5              # groups of 5 sub-steps
    LAST_SUBS = n_sub - 5 * (NG - 1)   # 1..5 sub-steps in the last group
    W = 5 * BL                         # 320: free width of one group block

    nc = bass.Bass("TRN2", debug=False, enable_asserts=False, num_devices=NCORES)

    # ---- DRAM I/O ----
    # pack1: [w_hhT | ident | w_roT | biasc] along free dim; pack2: w_ih row + b_ro col
    s_dram = nc.dram_tensor("s_seq", [1, T * BL], f32r, kind="ExternalInput").ap()
    pk1_dram = nc.dram_tensor("pack1", [N, 651], f32r, kind="ExternalInput").ap()
    pk2_dram = nc.dram_tensor("pack2", [N, 129], f32r, kind="ExternalInput").ap()
    out_dram = nc.dram_tensor("out", [10, BL], f32, kind="ExternalOutput").ap()

    with tile.TileContext(nc) as tc, ExitStack() as ctx:
        const_pool = ctx.enter_context(tc.tile_pool(name="const", bufs=1))
        state_pool = ctx.enter_context(tc.tile_pool(name="state", bufs=1))
        s_pool = ctx.enter_context(tc.tile_pool(name="schunk", bufs=2))
        psum_pool = ctx.enter_context(
            tc.tile_pool(name="psum", bufs=3, space="PSUM")
        )
        xacc_pool = ctx.enter_context(
            tc.tile_pool(name="xacc", bufs=1, space="PSUM")
        )

        # ---- constants (2 packed DMAs to stay under PE wait-slot limits) ----
        pk1 = const_pool.tile([N, 651], f32r)
        pk2 = const_pool.tile([N, 129], f32r)
        nc.sync.dma_start(pk1[:], pk1_dram)
        nc.sync.dma_start(pk2[:], pk2_dram)
        whh = pk1[:, 0:128]
        ident = pk1[:, 128:256]
        wro = pk1[:, 256:266].bitcast(f32)
        biasc = pk1[:, 266:267].bitcast(f32)
        wih = pk2[0:1, 0:128]
        bro = pk2[0:10, 128:129].bitcast(f32)

        # ---- per-group pool tiles ----
        # R_g holds group g's chain outputs [y_{5g+1} .. y_{5g+5}]; group g+1's
        # recurrent matmul consumes R_g wholesale (delay-4 window is exactly
        # the previous group's outputs).
        r_pool = ctx.enter_context(tc.tile_pool(name="r", bufs=3))
        th_pool = ctx.enter_context(tc.tile_pool(name="th", bufs=3))
        e_pool = ctx.enter_context(tc.tile_pool(name="e", bufs=3))
        f_pool = ctx.enter_context(tc.tile_pool(name="f", bufs=3))
        th0 = state_pool.tile([N, W], f32, name="th_init")
        rinit = state_pool.tile([N, W], f32, name="rinit")
        xacc_ps = xacc_pool.tile([N, W], f32)
        xacc_sb = state_pool.tile([N, BL], f32)
        out_ps = xacc_pool.tile([10, BL], f32)
        out_sb = state_pool.tile([10, BL], f32)
        # scratch PSUM target for 1x1 "touch" matmuls: PE matmuls only have a
        # single HW sync-wait slot, so a touch matmul absorbs each DMA-queue
        # semaphore before the real matmuls need the data
        touch_ps = xacc_pool.tile([1, 1], f32)

        def pe_touch(ap_1x1):
            a = ap_1x1.bitcast(f32)
            nc.tensor.matmul(
                touch_ps[:], a, a, start=True, stop=True,
                skip_group_check=True,
            )

        # zero-init: R_{-1} = [y_{-4}..y_0] = 0 and th_{-1} = 0.
        # rinit zeros come via DMA from pack1's zero region: walrus rejects
        # float32r Memset (ISA check), and the fp32r-matmul verifier rejects
        # f32-Memset producers; DMACopy with f32r output passes both.
        r32 = (lambda ap: ap.bitcast(f32r)) if use_f32r else (lambda ap: ap)
        nc.sync.dma_start(r32(rinit[:]), pk1_dram[:, 267:587])
        nc.vector.memset(th0[:, 4 * BL : W], 0.0)

        inv = float(1.0 / np.sqrt(N))

        schunk = None
        thp = th0
        rprev = rinit
        for g in range(NG):
            subs = 5 if g < NG - 1 else LAST_SUBS
            w = subs * BL  # th/e/f width needed this group
            thc = th_pool.tile([N, W], f32, name="th")
            E = e_pool.tile([N, W], f32, name="E")
            F = f_pool.tile([N, W], f32, name="F")
            R = r_pool.tile([N, W], f32, name="R")

            # stream the input sequence in chunks (rows 5g..: CHUNK_G groups)
            if g % CHUNK_G == 0:
                r0 = 5 * g * BL
                r1 = min(T * BL, (5 * g + 5 * CHUNK_G) * BL)
                schunk = s_pool.tile([1, 5 * CHUNK_G * BL], f32r)
                nc.sync.dma_start(schunk[:1, 0 : r1 - r0], s_dram[0:1, r0:r1])
                if g == 0:
                    pe_touch(pk1[0:1, 0:1])
                    pe_touch(pk2[0:1, 0:1])
                pe_touch(schunk[0:1, 0:1])
            soff = (g % CHUNK_G) * W

            # ---- PE: I = w_ih (x) s + w_hh @ y_delayed  (PSUM accumulate) ----
            # Rank-1 matmul covers the whole group at once (depends only on
            # the DMA'd input chunk).  The w_hh matmul is split per 64-column
            # block: block J only needs R_{g-1}[J] (chain output J of the
            # PREVIOUS group), so the tanh/e/f pipeline for group g streams
            # behind group g-1's chain instead of serializing after it.
            mm = psum_pool.tile([N, W], f32)
            nc.tensor.matmul(
                mm[:, 0:w],
                wih,
                schunk[0:1, soff : soff + w],
                start=True,
                stop=False,
                skip_group_check=True,
            )
            for J in range(subs):
                lo = J * BL
                nc.tensor.matmul(
                    mm[:, lo : lo + BL],
                    whh,
                    r32(rprev[:, lo : lo + BL]),
                    start=False,
                    stop=True,
                    skip_group_check=True,
                )
                # ---- ACT: th_J = tanh(inv * I_J + biasc) ----
                nc.scalar.activation(
                    thc[:, lo : lo + BL], mm[:, lo : lo + BL], Act.Tanh,
                    bias=biasc, scale=inv,
                )
                # ---- Pool: e_J = th_J - th_{J-1} ----
                if J == 0:
                    nc.gpsimd.tensor_sub(
                        E[:, 0:BL], thc[:, 0:BL], thp[:, 4 * BL : W]
                    )
                else:
                    nc.gpsimd.tensor_sub(
                        E[:, lo : lo + BL], thc[:, lo : lo + BL],
                        thc[:, lo - BL : lo],
                    )

            # ---- sub-steps: J produces y_{5g+J+1} -> R block J ----
            for J in range(subs):
                lo = J * BL
                # y_{t-1} = y_{5g+J-1}
                if J == 0:
                    ym1 = rprev[:, 3 * BL : 4 * BL]
                elif J == 1:
                    ym1 = rprev[:, 4 * BL : W]
                else:
                    ym1 = R[:, (J - 2) * BL : (J - 1) * BL]
                # y_t = y_{5g+J}
                yt = rprev[:, 4 * BL : W] if J == 0 else R[:, (J - 1) * BL : J * BL]
                ydst = R[:, lo : lo + BL]

                # DVE: f_t = -1.8*y_{t-1} + e_t  (scalar_tensor_tensor is
                # DVE-only in this walrus build)
                nc.vector.scalar_tensor_tensor(
                    F[:, lo : lo + BL], ym1, -1.8, E[:, lo : lo + BL],
                    op0=Alu.mult, op1=Alu.add,
                )
                # DVE (serial chain): y_{t+1} = 1.8*y_t + f_t
                # (write as f32r: PE consumes R in single-pass fp32r mode)
                nc.vector.scalar_tensor_tensor(
                    r32(ydst), yt, 1.8, F[:, lo : lo + BL],
                    op0=Alu.mult, op1=Alu.add,
                )

            thp = thc

            # ---- PE: x accumulation, xacc_ps[:, J*BL+b] += y_{5g+J+1}[b] ----
            nc.tensor.matmul(
                xacc_ps[:, 0:w],
                ident,
                r32(R[:, 0:w]),
                start=(g == 0),
                stop=(g == NG - 1),
                skip_group_check=True,
            )
            rprev = R

        # ---- epilogue: x_T = fold(xacc), out = w_ro @ x_T + b_ro ----
        nc.vector.tensor_reduce(
            xacc_sb[:],
            xacc_ps[:].rearrange("p (j b) -> p b j", j=5),
            axis=mybir.AxisListType.X,
            op=Alu.add,
        )
        nc.tensor.matmul(out_ps[:], wro, xacc_sb[:], start=True, stop=True)
        nc.vector.tensor_scalar(
            out_sb[:], out_ps[:], bro, None, op0=Alu.add
        )
        nc.sync.dma_start(out_dram, out_sb[:])

    _split_excess_waits(nc)
    return nc


def _split_excess_waits(nc):
    """This toolchain's walrus codegen accepts at most ONE semaphore wait per
    engine instruction ("Too many sync wait commands").  Tile emits up to ~3.
    Hoist all but the last wait of each instruction onto same-engine NoOps
    placed immediately before it -- the engine executes them in order, so the
    happens-before relation is preserved."""
    import concourse.mybir as mybir

    k = 0
    for fn in nc.m.functions:
        for bb in fn.blocks:
            insts = bb.instructions
            out = []
            changed = False
            for inst in insts:
                si = inst.sync_info
                waits = list(si.on_wait) if (si is not None and si.on_wait) else []
                if len(waits) > 1:
                    for wsub in waits[:-1]:
                        k += 1
                        nop = mybir.InstNoOp(name=f"WSPLIT-{k}", ins=[], outs=[])
                        nop.engine = inst.engine
                        nop.sync_info = mybir.SyncInfo(on_wait=[wsub], on_update=[])
                        out.append(nop)
                    inst.sync_info = mybir.SyncInfo(
                        on_wait=[waits[-1]], on_update=list(si.on_update or [])
                    )
                    changed = True
                out.append(inst)
            if changed:
                bb.instructions[:] = out


_BUILD_CACHE = {}


def _get_nc(T, use_f32r=True):
    key = (T, use_f32r)
    if key not in _BUILD_CACHE:
        _BUILD_CACHE[key] = _build(T, use_f32r)
    return _BUILD_CACHE[key]


def _run(input_sequence, w_ih, b_ih, w_hh, b_hh, w_ro, b_ro, T=None, trace=False, use_f32r=True):
    from concourse.bass_utils import run_bass_kernel_spmd

    input_sequence = np.asarray(input_sequence, np.float32)
    B, T_in = input_sequence.shape
    if T is None:
        T = T_in
    assert B == BL * NCORES
    nc = _get_nc(T, use_f32r)

    inv = np.float32(1.0 / np.sqrt(N))
    biasc_col = (
        (np.asarray(b_ih, np.float32) + np.asarray(b_hh, np.float32)) * inv
    )[:, None]
    pack1 = np.concatenate(
        [
            np.asarray(w_hh, np.float32).T,
            np.eye(N, dtype=np.float32),
            np.asarray(w_ro, np.float32).T,
            biasc_col,
            np.zeros((N, 384), np.float32),
        ],
        axis=1,
    ).astype(np.float32)
    pack2 = np.zeros((N, 129), np.float32)
    pack2[0, 0:128] = np.asarray(w_ih, np.float32)
    pack2[0:10, 128] = np.asarray(b_ro, np.float32)
    common = {"pack1": np.ascontiguousarray(pack1), "pack2": np.ascontiguousarray(pack2)}
    in_maps = []
    for i in range(NCORES):
        s_i = np.ascontiguousarray(
            input_sequence[i * BL : (i + 1) * BL, :T].T
        )  # [T, BL]
        m = dict(common)
        m["s_seq"] = s_i.reshape(1, T * BL)
        in_maps.append(m)

    res = run_bass_kernel_spmd(
        nc, in_maps, core_ids=list(range(NCORES)), trace=trace
    )
    outs = [r["out"] for r in res.results]  # each [10, 64]
    full = np.concatenate(outs, axis=1).T.astype(np.float32)  # [512, 10]
    return full, res


def kernel(input_sequence, w_ih, b_ih, w_hh, b_hh, w_ro, b_ro):
    out, _ = _run(input_sequence, w_ih, b_ih, w_hh, b_hh, w_ro, b_ro)
    return out
